# revision 15
# baseline (speedup 1.0000x reference)
"""Trainium2 Bass kernel for nn_ClarityTexture (clarity/texture enhancement).

Reference math (per image):
  L = 0.2126 R + 0.7152 G + 0.0722 B
  mid  = L - blur31(L)   (separable 31-tap gaussian, sigma 8, zero-padded)
  high = L - blur7(L)    (separable 7-tap gaussian, sigma 1.5)
  enhanced = L + tanh(clarity)*0.5*mid + tanh(texture)*0.3*high
  ratio = (enhanced + 1e-6) / (L + 1e-6)
  out = clip(x * ratio, 0, 1)

Sharding: pure data parallel — batch dim (8 images) across 8 NeuronCores.

Device algorithm per core (all matmuls on TensorE, float32r):
  - L via 3 accumulating diag matmuls (w_c * I stationary) per tile.
  - Pass 1 (vertical blur + transpose, fused): stationary = L chunk [rows,
    cols]; moving = band matrix B[k, nl] = g[k - nl + 2*halo]; output lands
    TRANSPOSED (V31T/V7T strips, cols on partitions) in PSUM, accumulated
    over row-chunks with overlapping windows (per-element has_written).
  - Pass 2 (horizontal blur + transpose back, fused): stationary = V*T
    chunk; moving = a1/a2-scaled band; S = a1*G31 + a2*G7 lands in ORIGINAL
    orientation in PSUM.
  - num = (L*a0 + eps) + S   (custom DVE affine_then_add, S read from PSUM)
  - rcp = Reciprocal(L + eps)  (ScalarE LUT)
  - ratio = num * rcp; out_c = min(max(ratio,0)*x_c, 1)  (VectorE)

a0/a1/a2 depend on the runtime scalars clarity/texture, so they are shipped
as inputs (a1/a2 baked into the pass-2 band matrices, a0 as a [128,1]
per-partition scalar) — the compiled NEFF is input-independent and cached.
"""

from contextlib import ExitStack

import numpy as np
import ml_dtypes

import concourse.bass as bass
import concourse.tile as tile
import concourse.mybir as mybir
from concourse import bacc, bass_utils

F32 = mybir.dt.float32
F32R = mybir.dt.float32r
BF16 = mybir.dt.bfloat16

H = W = 1024
P = 128
NT = H // P          # 8 row/col chunks
BANK = 512           # PSUM bank, fp32 elements
EPS = 1e-6
N_CORES = 8
LW = (0.2126, 0.7152, 0.0722)
HALO31, HALO7 = 15, 3
NW31, NW7 = P + 2 * HALO31, P + 2 * HALO7   # 158, 134


def _gauss1d(size, sigma):
    grid = np.arange(size, dtype=np.float32) - size // 2
    g = np.exp(-(grid ** 2) / (2.0 * sigma * sigma))
    return (g / g.sum()).astype(np.float32)


def _band_moving(g, halo):
    """B[k, nl] = g[k - nl + 2*halo], shape [128, 128 + 2*halo]."""
    taps = g.shape[0]
    B = np.zeros((P, P + 2 * halo), np.float32)
    for k in range(P):
        lo = max(0, k)          # nl range where 0 <= k-nl+2h < taps
        hi = min(P + 2 * halo, k + taps)
        for nl in range(lo, hi):
            B[k, nl] = g[k - nl + 2 * halo]
    return B


def _window_segments(q, halo):
    """Chunk q's output window [128q-halo, 128q+127+halo] clipped to
    [0, 1024), split at 512 boundaries.
    Yields (bank, out_lo, out_len, nl_lo) with out_lo local to the bank."""
    r0 = P * q - halo
    w_lo, w_hi = max(0, r0), min(H, P * q + P + halo)
    s = w_lo
    while s < w_hi:
        e = min(w_hi, (s // BANK + 1) * BANK)
        yield (s // BANK, s - (s // BANK) * BANK, e - s, s - r0)
        s = e


def _acc_plan(halo):
    """Per bank: ordered list over chunks q of segments, with first/last
    flags for PSUM start/stop."""
    per_bank = {0: [], 1: []}
    for q in range(NT):
        for (b, o, ln, n) in _window_segments(q, halo):
            per_bank[b].append((q, o, ln, n))
    return per_bank


class _Cache:
    runners = {}


def _build_kernel(reps=1):
    nc = bacc.Bacc("TRN2", target_bir_lowering=False, debug=False,
                   num_devices=N_CORES)

    x_d = nc.dram_tensor("x", [3, H, W], F32, kind="ExternalInput").ap()
    out_d = nc.dram_tensor("out", [3, H, W], F32, kind="ExternalOutput").ap()
    # [B31M (158) | B7M (134)] unscaled, for pass 1
    bm1_d = nc.dram_tensor("bm1", [P, NW31 + NW7], F32, kind="ExternalInput").ap()
    # a1-scaled B31M for pass 2
    bm2_d = nc.dram_tensor("bm2", [P, NW31], F32, kind="ExternalInput").ap()
    # a2-scaled B7M for pass 2 (bf16 path)
    bm2b_d = nc.dram_tensor("bm2b", [P, NW7], BF16, kind="ExternalInput").ap()
    # [w0*I | w1*I | w2*I]
    dw_d = nc.dram_tensor("dw", [P, 3 * P], F32, kind="ExternalInput").ap()
    # a0 per-partition scalar, and b0 = eps*(1-a0)
    a0_d = nc.dram_tensor("a0", [P, 1], F32, kind="ExternalInput").ap()
    b0_d = nc.dram_tensor("b0", [P, 1], F32, kind="ExternalInput").ap()

    plan31 = _acc_plan(HALO31)
    plan7 = _acc_plan(HALO7)

    with tile.TileContext(nc) as tc, ExitStack() as ctx:
        cpool = ctx.enter_context(tc.tile_pool(name="consts", bufs=1))
        bm1 = cpool.tile([P, NW31 + NW7], F32)
        nc.sync.dma_start(bm1[:], bm1_d[:])
        bm2 = cpool.tile([P, NW31], F32)
        nc.sync.dma_start(bm2[:], bm2_d[:])
        bm2b = cpool.tile([P, NW7], BF16)
        nc.sync.dma_start(bm2b[:], bm2b_d[:])
        dw = cpool.tile([P, 3 * P], F32)
        nc.sync.dma_start(dw[:], dw_d[:])
        a0t = cpool.tile([P, 1], F32)
        nc.sync.dma_start(a0t[:], a0_d[:])
        b0t = cpool.tile([P, 1], F32)
        nc.sync.dma_start(b0t[:], b0_d[:])

        bigpool = ctx.enter_context(tc.tile_pool(name="planes", bufs=1))
        X = bigpool.tile([P, 3 * NT * H], F32)      # 12 MB, (c,i) plane at (c*8+i)*1024
        L = bigpool.tile([P, NT * H], F32)          # 4 MB, tile i at 1024*i
        V31T = bigpool.tile([P, NT * H], F32)       # 4 MB, strip j at 1024*j
        V7T = bigpool.tile([P, NT * H], BF16)       # 2 MB

        work = ctx.enter_context(tc.tile_pool(name="work", bufs=2))
        ostage = ctx.enter_context(tc.tile_pool(name="ostage", bufs=4))

        for rep in range(reps):
            _emit_body(nc, tc, ctx, x_d, out_d, bm1, bm2, bm2b, dw, a0t, b0t,
                       X, L, V31T, V7T, work, ostage, plan31, plan7)

    nc.compile()
    return nc


def _emit_body(nc, tc, ctx, x_d, out_d, bm1, bm2, bm2b, dw, a0t, b0t,
               X, L, V31T, V7T, work, ostage, plan31, plan7):
    if True:
        # ---- load x ----
        for c in range(3):
            for i in range(NT):
                nc.sync.dma_start(X[:, (c * NT + i) * H:(c * NT + i + 1) * H],
                                  x_d[c, i * P:(i + 1) * P, :])

        # ---- luma ----
        with tc.tile_pool(name="lpsum", bufs=4, space="PSUM") as lpsum:
            for i in range(NT):
                for h in range(2):
                    lp = lpsum.tile([P, BANK], F32)
                    for c in range(3):
                        nc.tensor.matmul(
                            lp[:],
                            dw[:, c * P:(c + 1) * P],
                            X[:, (c * NT + i) * H + h * BANK:
                               (c * NT + i) * H + (h + 1) * BANK],
                            start=(c == 0), stop=(c == 2))
                    nc.scalar.activation(
                        L[:, i * H + h * BANK:i * H + (h + 1) * BANK], lp[:],
                        mybir.ActivationFunctionType.Copy, bias=EPS, scale=1.0)

        # ---- pass 1: fused vertical blur + transpose ----
        vtp = tc.tile_pool(name="vtpsum", bufs=2, space="PSUM")
        vtpsum = vtp.__enter__()
        for j in range(NT):
            p31 = [vtpsum.tile([P, BANK], F32, name="p31", tag="p31") for _ in range(2)]
            p7 = [vtpsum.tile([P, BANK], F32, name="p7", tag="p7") for _ in range(2)]
            for plan, ptiles, nw_off in ((plan31, p31, 0), (plan7, p7, NW31)):
                for b in range(2):
                    segs = plan[b]
                    for si, (q, o, ln, n) in enumerate(segs):
                        lhsT = L[:, q * H + j * P:q * H + j * P + P]
                        nc.tensor.matmul(
                            ptiles[b][:, o:o + ln],
                            lhsT,
                            bm1[:, nw_off + n:nw_off + n + ln],
                            start=(si == 0), stop=(si == len(segs) - 1))
            for b in range(2):
                nc.scalar.copy(
                    V31T[:, j * H + b * BANK:j * H + (b + 1) * BANK], p31[b][:])
                nc.scalar.copy(
                    V7T[:, j * H + b * BANK:j * H + (b + 1) * BANK], p7[b][:])

        vtp.__exit__(None, None, None)

        # ---- pass 2 + elementwise tail, per output row-chunk i ----
        spp = tc.tile_pool(name="spsum", bufs=2, space="PSUM")
        spsum = spp.__enter__()
        for i in range(NT):
            sp = [spsum.tile([P, BANK], F32, name="sp", tag="sp") for _ in range(2)]
            # first/last bookkeeping across BOTH blurs per bank
            order = {0: [], 1: []}
            for j in range(NT):
                for (b, o, ln, n) in _window_segments(j, HALO31):
                    order[b].append(("31", j, o, ln, n))
                for (b, o, ln, n) in _window_segments(j, HALO7):
                    order[b].append(("7", j, o, ln, n))
            # emit in j order (stationary reuse), flags from position
            for b in range(2):
                segs = order[b]
                for si, (kind, j, o, ln, n) in enumerate(segs):
                    if kind == "31":
                        lhsT = V31T[:, j * H + i * P:j * H + i * P + P]
                        rhs = bm2[:, n:n + ln]
                    else:
                        lhsT = V7T[:, j * H + i * P:j * H + i * P + P]
                        rhs = bm2b[:, n:n + ln]
                    nc.tensor.matmul(sp[b][:, o:o + ln], lhsT, rhs,
                                     start=(si == 0), stop=(si == len(segs) - 1))
            for h in range(2):
                lsl = L[:, i * H + h * BANK:i * H + (h + 1) * BANK]
                num = work.tile([P, BANK], F32, name="num", tag="num")
                nc.vector.affine_then_add(num[:], lsl, sp[h][:],
                                          scale=a0t[:, 0:1], bias=b0t[:, 0:1])
                # rcp = 1/den via exp(-ln(den)); den = L' = L + eps > 0
                lnt = work.tile([P, BANK], F32, name="lnt", tag="lnt")
                nc.scalar.activation(lnt[:], lsl,
                                     mybir.ActivationFunctionType.Ln)
                rcp = work.tile([P, BANK], F32, name="rcp", tag="rcp")
                nc.scalar.activation(rcp[:], lnt[:],
                                     mybir.ActivationFunctionType.Exp,
                                     scale=-1.0)
                # ratio0 = max(num,0)*rcp  (rcp>0, so this == max(ratio,0))
                ratio = work.tile([P, BANK], F32, name="ratio", tag="ratio")
                nc.vector.scalar_tensor_tensor(
                    ratio[:], num[:], 0.0, rcp[:],
                    mybir.AluOpType.max, mybir.AluOpType.mult)
                for c in range(3):
                    y = ostage.tile([P, BANK], F32, name="y", tag="y")
                    nc.vector.tensor_tensor(
                        y[:], ratio[:],
                        X[:, (c * NT + i) * H + h * BANK:
                           (c * NT + i) * H + (h + 1) * BANK],
                        mybir.AluOpType.mult)
                    nc.vector.tensor_scalar_min(y[:], y[:], 1.0)
                    nc.sync.dma_start(
                        out_d[c, i * P:(i + 1) * P, h * BANK:(h + 1) * BANK],
                        y[:])
        spp.__exit__(None, None, None)


def _host_inputs(clarity, texture):
    g31 = _gauss1d(31, 8.0)
    g7 = _gauss1d(7, 1.5)
    ca = np.float32(np.tanh(np.float32(clarity))) * np.float32(0.5)
    ta = np.float32(np.tanh(np.float32(texture))) * np.float32(0.3)
    a0 = np.float32(1.0) + ca + ta
    b31 = _band_moving(g31, HALO31)
    b7 = _band_moving(g7, HALO7)
    dw = np.zeros((P, 3 * P), np.float32)
    for c in range(3):
        dw[:, c * P:(c + 1) * P] = np.float32(LW[c]) * np.eye(P, dtype=np.float32)
    return {
        "bm1": np.concatenate([b31, b7], axis=1),
        "bm2": (-ca) * b31,
        "bm2b": ((-ta) * b7).astype(ml_dtypes.bfloat16),
        "dw": dw,
        "a0": np.full((P, 1), a0, np.float32),
        "b0": np.full((P, 1), np.float32(EPS) * (np.float32(1.0) - a0), np.float32),
    }


class _Runner:
    """Cached jitted executable for the SPMD NEFF (mirrors
    bass2jax.run_bass_via_pjrt but reuses the jitted fn across calls)."""

    def __init__(self, nc):
        import jax
        from jax.sharding import Mesh, PartitionSpec
        from jax.experimental.shard_map import shard_map
        import concourse.mybir as _mybir
        from concourse import bass2jax
        from concourse.bass2jax import _bass_exec_p, install_neuronx_cc_hook

        install_neuronx_cc_hook()
        self.nc = nc
        in_names, out_names, out_avals, zero_outs = [], [], [], []
        for alloc in nc.m.functions[0].allocations:
            if not isinstance(alloc, _mybir.MemoryLocationSet):
                continue
            name = alloc.memorylocations[0].name
            if alloc.kind == "ExternalInput":
                in_names.append(name)
            elif alloc.kind == "ExternalOutput":
                shape = tuple(alloc.tensor_shape)
                dtype = _mybir.dt.np(alloc.dtype)
                out_names.append(name)
                out_avals.append(jax.core.ShapedArray(shape, dtype))
                zero_outs.append(np.zeros(shape, dtype))
        self.in_names = list(in_names)
        self.out_names = out_names
        self.out_avals = out_avals
        self.zero_outs = zero_outs
        n_params = len(self.in_names)
        all_in_names = tuple(self.in_names + out_names)

        def _body(*args):
            outs = _bass_exec_p.bind(
                *args,
                out_avals=tuple(out_avals),
                in_names=all_in_names,
                out_names=tuple(out_names),
                lowering_input_output_aliases=(),
                sim_require_finite=True,
                sim_require_nnan=True,
                nc=nc,
            )
            return tuple(outs)

        devices = jax.devices()[:N_CORES]
        mesh = Mesh(np.asarray(devices), ("core",))
        n_outs = len(out_names)
        in_specs = (PartitionSpec("core"),) * (n_params + n_outs)
        out_specs = (PartitionSpec("core"),) * n_outs
        donate = tuple(range(n_params, n_params + n_outs))
        self.fn = jax.jit(
            shard_map(_body, mesh=mesh, in_specs=in_specs,
                      out_specs=out_specs, check_rep=False),
            donate_argnums=donate, keep_unused=True)

    def __call__(self, in_maps):
        n = len(in_maps)
        concat_in = [
            np.concatenate([np.asarray(in_maps[c][name]) for c in range(n)],
                           axis=0)
            for name in self.in_names
        ]
        concat_zeros = [
            np.zeros((n * z.shape[0], *z.shape[1:]), z.dtype)
            for z in self.zero_outs
        ]
        out_arrs = self.fn(*concat_in, *concat_zeros)
        return [
            {name: np.asarray(out_arrs[i]).reshape(
                n, *self.out_avals[i].shape)[c]
             for i, name in enumerate(self.out_names)}
            for c in range(n)
        ]


def _get_runner(reps=1):
    if reps not in _Cache.runners:
        _Cache.runners[reps] = _Runner(_build_kernel(reps))
    return _Cache.runners[reps]


def _make_in_maps(x, clarity, texture):
    consts = _host_inputs(clarity, texture)
    return [{"x": x[i], **consts} for i in range(N_CORES)]


def kernel(x, clarity, texture):
    x = np.ascontiguousarray(np.asarray(x, np.float32))
    assert x.shape == (N_CORES, 3, H, W)
    res = _get_runner(1)(_make_in_maps(x, clarity, texture))
    return np.stack([r["out"] for r in res]).astype(np.float32)


# revision 18
# speedup vs baseline: 14756.0102x; 14756.0102x over previous
"""Trainium2 Bass kernel for nn_ClarityTexture (clarity/texture enhancement).

Reference math (per image):
  L = 0.2126 R + 0.7152 G + 0.0722 B
  mid  = L - blur31(L)   (separable 31-tap gaussian, sigma 8, zero-padded)
  high = L - blur7(L)    (separable 7-tap gaussian, sigma 1.5)
  enhanced = L + tanh(clarity)*0.5*mid + tanh(texture)*0.3*high
  ratio = (enhanced + 1e-6) / (L + 1e-6)
  out = clip(x * ratio, 0, 1)

Sharding: pure data parallel — batch dim (8 images) across 8 NeuronCores.

Device algorithm per core (all matmuls on TensorE, float32r):
  - L via 3 accumulating diag matmuls (w_c * I stationary) per tile.
  - Pass 1 (vertical blur + transpose, fused): stationary = L chunk [rows,
    cols]; moving = band matrix B[k, nl] = g[k - nl + 2*halo]; output lands
    TRANSPOSED (V31T/V7T strips, cols on partitions) in PSUM, accumulated
    over row-chunks with overlapping windows (per-element has_written).
  - Pass 2 (horizontal blur + transpose back, fused): stationary = V*T
    chunk; moving = a1/a2-scaled band; S = a1*G31 + a2*G7 lands in ORIGINAL
    orientation in PSUM.
  - num = (L*a0 + eps) + S   (custom DVE affine_then_add, S read from PSUM)
  - rcp = Reciprocal(L + eps)  (ScalarE LUT)
  - ratio = num * rcp; out_c = min(max(ratio,0)*x_c, 1)  (VectorE)

a0/a1/a2 depend on the runtime scalars clarity/texture, so they are shipped
as inputs (a1/a2 baked into the pass-2 band matrices, a0 as a [128,1]
per-partition scalar) — the compiled NEFF is input-independent and cached.
"""

from contextlib import ExitStack

import numpy as np
import ml_dtypes

import concourse.bass as bass
import concourse.tile as tile
import concourse.mybir as mybir
from concourse import bacc, bass_utils

F32 = mybir.dt.float32
F32R = mybir.dt.float32r
BF16 = mybir.dt.bfloat16

H = W = 1024
P = 128
NT = H // P          # 8 row/col chunks
BANK = 512           # PSUM bank, fp32 elements
EPS = 1e-6
N_CORES = 8
LW = (0.2126, 0.7152, 0.0722)
HALO31, HALO7 = 15, 3
NW31, NW7 = P + 2 * HALO31, P + 2 * HALO7   # 158, 134


def _gauss1d(size, sigma):
    grid = np.arange(size, dtype=np.float32) - size // 2
    g = np.exp(-(grid ** 2) / (2.0 * sigma * sigma))
    return (g / g.sum()).astype(np.float32)


def _band_moving(g, halo):
    """B[k, nl] = g[k - nl + 2*halo], shape [128, 128 + 2*halo]."""
    taps = g.shape[0]
    B = np.zeros((P, P + 2 * halo), np.float32)
    for k in range(P):
        lo = max(0, k)          # nl range where 0 <= k-nl+2h < taps
        hi = min(P + 2 * halo, k + taps)
        for nl in range(lo, hi):
            B[k, nl] = g[k - nl + 2 * halo]
    return B


def _window_segments(q, halo):
    """Chunk q's output window [128q-halo, 128q+127+halo] clipped to
    [0, 1024), split at 512 boundaries.
    Yields (bank, out_lo, out_len, nl_lo) with out_lo local to the bank."""
    r0 = P * q - halo
    w_lo, w_hi = max(0, r0), min(H, P * q + P + halo)
    s = w_lo
    while s < w_hi:
        e = min(w_hi, (s // BANK + 1) * BANK)
        yield (s // BANK, s - (s // BANK) * BANK, e - s, s - r0)
        s = e


def _acc_plan(halo):
    """Per bank: ordered list over chunks q of segments, with first/last
    flags for PSUM start/stop."""
    per_bank = {0: [], 1: []}
    for q in range(NT):
        for (b, o, ln, n) in _window_segments(q, halo):
            per_bank[b].append((q, o, ln, n))
    return per_bank


class _Cache:
    runners = {}


def _build_kernel(reps=1):
    nc = bacc.Bacc("TRN2", target_bir_lowering=False, debug=False,
                   num_devices=N_CORES)

    x_d = nc.dram_tensor("x", [3, H, W], F32, kind="ExternalInput").ap()
    out_d = nc.dram_tensor("out", [3, H, W], F32, kind="ExternalOutput").ap()
    # [B31M (158) | B7M (134)] unscaled, for pass 1
    bm1_d = nc.dram_tensor("bm1", [P, NW31 + NW7], F32, kind="ExternalInput").ap()
    # a1-scaled B31M for pass 2
    bm2_d = nc.dram_tensor("bm2", [P, NW31], F32, kind="ExternalInput").ap()
    # a2-scaled B7M for pass 2 (bf16 path)
    bm2b_d = nc.dram_tensor("bm2b", [P, NW7], BF16, kind="ExternalInput").ap()
    # [w0*I | w1*I | w2*I]
    dw_d = nc.dram_tensor("dw", [P, 3 * P], F32, kind="ExternalInput").ap()
    # a0 per-partition scalar, and b0 = eps*(1-a0)
    a0_d = nc.dram_tensor("a0", [P, 1], F32, kind="ExternalInput").ap()
    b0_d = nc.dram_tensor("b0", [P, 1], F32, kind="ExternalInput").ap()

    plan31 = _acc_plan(HALO31)
    plan7 = _acc_plan(HALO7)

    with tile.TileContext(nc) as tc, ExitStack() as ctx:
        cpool = ctx.enter_context(tc.tile_pool(name="consts", bufs=1))
        bm1 = cpool.tile([P, NW31 + NW7], F32)
        nc.sync.dma_start(bm1[:], bm1_d[:])
        bm2 = cpool.tile([P, NW31], F32)
        nc.sync.dma_start(bm2[:], bm2_d[:])
        bm2b = cpool.tile([P, NW7], BF16)
        nc.sync.dma_start(bm2b[:], bm2b_d[:])
        dw = cpool.tile([P, 3 * P], F32)
        nc.sync.dma_start(dw[:], dw_d[:])
        a0t = cpool.tile([P, 1], F32)
        nc.sync.dma_start(a0t[:], a0_d[:])
        b0t = cpool.tile([P, 1], F32)
        nc.sync.dma_start(b0t[:], b0_d[:])

        bigpool = ctx.enter_context(tc.tile_pool(name="planes", bufs=1))
        X = bigpool.tile([P, 3 * NT * H], F32)      # 12 MB, (c,i) plane at (c*8+i)*1024
        L = bigpool.tile([P, NT * H], F32)          # 4 MB, tile i at 1024*i
        V31T = bigpool.tile([P, NT * H], F32)       # 4 MB, strip j at 1024*j
        V7T = bigpool.tile([P, NT * H], BF16)       # 2 MB

        work = ctx.enter_context(tc.tile_pool(name="work", bufs=2))
        ostage = ctx.enter_context(tc.tile_pool(name="ostage", bufs=4))

        for rep in range(reps):
            _emit_body(nc, tc, ctx, x_d, out_d, bm1, bm2, bm2b, dw, a0t, b0t,
                       X, L, V31T, V7T, work, ostage, plan31, plan7)

    nc.compile()
    return nc


def _emit_body(nc, tc, ctx, x_d, out_d, bm1, bm2, bm2b, dw, a0t, b0t,
               X, L, V31T, V7T, work, ostage, plan31, plan7):
    if True:
        # ---- load x ----
        for c in range(3):
            for i in range(NT):
                nc.sync.dma_start(X[:, (c * NT + i) * H:(c * NT + i + 1) * H],
                                  x_d[c, i * P:(i + 1) * P, :])

        # ---- luma ----
        with tc.tile_pool(name="lpsum", bufs=4, space="PSUM") as lpsum:
            for i in range(NT):
                for h in range(2):
                    lp = lpsum.tile([P, BANK], F32)
                    for c in range(3):
                        nc.tensor.matmul(
                            lp[:],
                            dw[:, c * P:(c + 1) * P],
                            X[:, (c * NT + i) * H + h * BANK:
                               (c * NT + i) * H + (h + 1) * BANK],
                            start=(c == 0), stop=(c == 2))
                    nc.scalar.activation(
                        L[:, i * H + h * BANK:i * H + (h + 1) * BANK], lp[:],
                        mybir.ActivationFunctionType.Copy, bias=EPS, scale=1.0)

        # ---- pass 1: fused vertical blur + transpose ----
        vtp = tc.tile_pool(name="vtpsum", bufs=2, space="PSUM")
        vtpsum = vtp.__enter__()
        for j in range(NT):
            p31 = [vtpsum.tile([P, BANK], F32, name="p31", tag="p31") for _ in range(2)]
            p7 = [vtpsum.tile([P, BANK], F32, name="p7", tag="p7") for _ in range(2)]
            for plan, ptiles, nw_off in ((plan31, p31, 0), (plan7, p7, NW31)):
                for b in range(2):
                    segs = plan[b]
                    for si, (q, o, ln, n) in enumerate(segs):
                        lhsT = L[:, q * H + j * P:q * H + j * P + P]
                        nc.tensor.matmul(
                            ptiles[b][:, o:o + ln],
                            lhsT,
                            bm1[:, nw_off + n:nw_off + n + ln],
                            start=(si == 0), stop=(si == len(segs) - 1))
            for b in range(2):
                nc.scalar.copy(
                    V31T[:, j * H + b * BANK:j * H + (b + 1) * BANK], p31[b][:])
                nc.scalar.copy(
                    V7T[:, j * H + b * BANK:j * H + (b + 1) * BANK], p7[b][:])

        vtp.__exit__(None, None, None)

        # ---- pass 2 + elementwise tail, per output row-chunk i ----
        spp = tc.tile_pool(name="spsum", bufs=2, space="PSUM")
        spsum = spp.__enter__()
        for i in range(NT):
            sp = [spsum.tile([P, BANK], F32, name="sp", tag="sp") for _ in range(2)]
            # first/last bookkeeping across BOTH blurs per bank
            order = {0: [], 1: []}
            for j in range(NT):
                for (b, o, ln, n) in _window_segments(j, HALO31):
                    order[b].append(("31", j, o, ln, n))
                for (b, o, ln, n) in _window_segments(j, HALO7):
                    order[b].append(("7", j, o, ln, n))
            # emit in j order (stationary reuse), flags from position
            for b in range(2):
                segs = order[b]
                for si, (kind, j, o, ln, n) in enumerate(segs):
                    if kind == "31":
                        lhsT = V31T[:, j * H + i * P:j * H + i * P + P]
                        rhs = bm2[:, n:n + ln]
                    else:
                        lhsT = V7T[:, j * H + i * P:j * H + i * P + P]
                        rhs = bm2b[:, n:n + ln]
                    nc.tensor.matmul(sp[b][:, o:o + ln], lhsT, rhs,
                                     start=(si == 0), stop=(si == len(segs) - 1))
            for h in range(2):
                lsl = L[:, i * H + h * BANK:i * H + (h + 1) * BANK]
                num = work.tile([P, BANK], F32, name="num", tag="num")
                nc.vector.affine_then_add(num[:], lsl, sp[h][:],
                                          scale=a0t[:, 0:1], bias=b0t[:, 0:1])
                # rcp = 1/den via exp(-ln(den)); den = L' = L + eps > 0
                lnt = work.tile([P, BANK], F32, name="lnt", tag="lnt")
                nc.scalar.activation(lnt[:], lsl,
                                     mybir.ActivationFunctionType.Ln)
                rcp = work.tile([P, BANK], F32, name="rcp", tag="rcp")
                nc.scalar.activation(rcp[:], lnt[:],
                                     mybir.ActivationFunctionType.Exp,
                                     scale=-1.0)
                # ratio0 = max(num,0)*rcp  (rcp>0, so this == max(ratio,0))
                ratio = work.tile([P, BANK], F32, name="ratio", tag="ratio")
                nc.vector.scalar_tensor_tensor(
                    ratio[:], num[:], 0.0, rcp[:],
                    mybir.AluOpType.max, mybir.AluOpType.mult)
                for c in range(3):
                    y = ostage.tile([P, BANK], F32, name="y", tag="y")
                    nc.vector.tensor_tensor(
                        y[:], ratio[:],
                        X[:, (c * NT + i) * H + h * BANK:
                           (c * NT + i) * H + (h + 1) * BANK],
                        mybir.AluOpType.mult)
                    nc.vector.tensor_scalar_min(y[:], y[:], 1.0)
                    nc.sync.dma_start(
                        out_d[c, i * P:(i + 1) * P, h * BANK:(h + 1) * BANK],
                        y[:])
        spp.__exit__(None, None, None)


def _host_inputs(clarity, texture):
    g31 = _gauss1d(31, 8.0)
    g7 = _gauss1d(7, 1.5)
    ca = np.float32(np.tanh(np.float32(clarity))) * np.float32(0.5)
    ta = np.float32(np.tanh(np.float32(texture))) * np.float32(0.3)
    a0 = np.float32(1.0) + ca + ta
    b31 = _band_moving(g31, HALO31)
    b7 = _band_moving(g7, HALO7)
    dw = np.zeros((P, 3 * P), np.float32)
    for c in range(3):
        dw[:, c * P:(c + 1) * P] = np.float32(LW[c]) * np.eye(P, dtype=np.float32)
    return {
        "bm1": np.concatenate([b31, b7], axis=1),
        "bm2": (-ca) * b31,
        "bm2b": ((-ta) * b7).astype(ml_dtypes.bfloat16),
        "dw": dw,
        "a0": np.full((P, 1), a0, np.float32),
        "b0": np.full((P, 1), np.float32(EPS) * (np.float32(1.0) - a0), np.float32),
    }


class _Runner:
    """Cached jitted executable for the SPMD NEFF (mirrors
    bass2jax.run_bass_via_pjrt but reuses the jitted fn across calls)."""

    def __init__(self, nc):
        import jax
        from jax.sharding import Mesh, PartitionSpec
        from jax.experimental.shard_map import shard_map
        import concourse.mybir as _mybir
        from concourse import bass2jax
        from concourse.bass2jax import _bass_exec_p, install_neuronx_cc_hook

        install_neuronx_cc_hook()
        self.nc = nc
        partition_name = (nc.partition_id_tensor.name
                          if nc.partition_id_tensor else None)
        in_names, out_names, out_avals, zero_outs = [], [], [], []
        for alloc in nc.m.functions[0].allocations:
            if not isinstance(alloc, _mybir.MemoryLocationSet):
                continue
            name = alloc.memorylocations[0].name
            if alloc.kind == "ExternalInput":
                if name != partition_name:
                    in_names.append(name)
            elif alloc.kind == "ExternalOutput":
                shape = tuple(alloc.tensor_shape)
                dtype = _mybir.dt.np(alloc.dtype)
                out_names.append(name)
                out_avals.append(jax.core.ShapedArray(shape, dtype))
                zero_outs.append(np.zeros(shape, dtype))
        self.in_names = list(in_names)
        self.out_names = out_names
        self.out_avals = out_avals
        self.zero_outs = zero_outs
        n_params = len(self.in_names)
        all_in_names = list(self.in_names) + list(out_names)
        if partition_name is not None:
            all_in_names.append(partition_name)
        all_in_names = tuple(all_in_names)

        def _body(*args):
            operands = list(args)
            if partition_name is not None:
                operands.append(bass2jax.partition_id_tensor())
            outs = _bass_exec_p.bind(
                *operands,
                out_avals=tuple(out_avals),
                in_names=all_in_names,
                out_names=tuple(out_names),
                lowering_input_output_aliases=(),
                sim_require_finite=True,
                sim_require_nnan=True,
                nc=nc,
            )
            return tuple(outs)

        devices = jax.devices()[:N_CORES]
        self.mesh = Mesh(np.asarray(devices), ("core",))
        n_outs = len(out_names)
        in_specs = (PartitionSpec("core"),) * (n_params + n_outs)
        out_specs = (PartitionSpec("core"),) * n_outs
        donate = tuple(range(n_params, n_params + n_outs))
        mapped = shard_map(_body, mesh=self.mesh, in_specs=in_specs,
                           out_specs=out_specs, check_rep=False)
        self.fn = jax.jit(mapped, donate_argnums=donate, keep_unused=True)
        # no-donate variant for repeated timing with device-resident args
        self.fn_nodonate = jax.jit(mapped, keep_unused=True)
        self.in_sharding = jax.sharding.NamedSharding(
            self.mesh, PartitionSpec("core"))

    def __call__(self, in_maps):
        n = len(in_maps)
        concat_in = [
            np.concatenate([np.asarray(in_maps[c][name]) for c in range(n)],
                           axis=0)
            for name in self.in_names
        ]
        concat_zeros = [
            np.zeros((n * z.shape[0], *z.shape[1:]), z.dtype)
            for z in self.zero_outs
        ]
        out_arrs = self.fn(*concat_in, *concat_zeros)
        return [
            {name: np.asarray(out_arrs[i]).reshape(
                n, *self.out_avals[i].shape)[c]
             for i, name in enumerate(self.out_names)}
            for c in range(n)
        ]

    def device_args(self, in_maps):
        """device_put the concatenated inputs+zeros once, for timing."""
        import jax
        n = len(in_maps)
        concat_in = [
            np.concatenate([np.asarray(in_maps[c][name]) for c in range(n)],
                           axis=0)
            for name in self.in_names
        ]
        concat_zeros = [
            np.zeros((n * z.shape[0], *z.shape[1:]), z.dtype)
            for z in self.zero_outs
        ]
        return [jax.device_put(a, self.in_sharding)
                for a in (*concat_in, *concat_zeros)]

    def run_device(self, dargs):
        """no-donate, device-resident call; returns unfetched jax arrays."""
        out = self.fn_nodonate(*dargs)
        for o in out:
            o.block_until_ready()
        return out


def _get_runner(reps=1):
    if reps not in _Cache.runners:
        _Cache.runners[reps] = _Runner(_build_kernel(reps))
    return _Cache.runners[reps]


def _make_in_maps(x, clarity, texture):
    consts = _host_inputs(clarity, texture)
    return [{"x": x[i], **consts} for i in range(N_CORES)]


def kernel(x, clarity, texture):
    x = np.ascontiguousarray(np.asarray(x, np.float32))
    assert x.shape == (N_CORES, 3, H, W)
    res = _get_runner(1)(_make_in_maps(x, clarity, texture))
    return np.stack([r["out"] for r in res]).astype(np.float32)
